# revision 6
# baseline (speedup 1.0000x reference)
"""BitLinear forward kernel for Trainium2 (8 NeuronCores) — v9.

v9 reworks v8 around the measured bottlenecks:
- x is transposed on the HOST (layout-only), so the device gets xT [D, T]
  f32 and the 256 PE transposes + psum-eviction pipeline of the x phase
  disappear.
- x streams in 4 token-strips of 512 over the two hardware DGE rings
  (scalar + sync), sign-quantized by a single DVE pass to {-0.5,+0.5}
  fp8 (the missing 2x is folded into alpha).
- W quant is sharded as before (2 blocks of 128 rows/core); each block's
  ternary code is PE-transposed, AllGathered early (fp8 payload + alpha),
  and bounce-read per chunk so the first 8 output blocks can matmul
  while the second AllGather is still in flight.
- mm: stationary code chunk, moving sign(x)^T, fp8 DoubleRow, psum
  [128,512] accumulated over 8 DoubleRow d-pairs; ACT evicts with
  per-partition scale alpha' = -2*alpha (+ bias) straight to bf16 and
  the y^T strip DMAs out immediately.
"""

import sys

for _p in ("/opt/trn_rl_repo", "/opt/trn_rl_repo/concourse"):
    if _p not in sys.path:
        sys.path.insert(0, _p)

import numpy as np

import concourse.bass as bass
import concourse.tile as tile
import concourse.mybir as mybir
from concourse import bacc
from concourse.bass_utils import run_bass_kernel_spmd
from concourse.masks import make_identity

# Problem shape (hardcoded per contract)
B, S, D, O = 4, 4096, 2048, 2048
N_CORES = 8
T = (B * S) // N_CORES  # 2048 token rows per core
OSH = O // N_CORES      # 256 W rows quantized per core
DELTA_W = 0.05

P = 128
NWL = OSH // P  # 2 local W blocks
ND = D // P     # 16 d-tiles
DP = ND // 2    # 8 DoubleRow d-pairs
NOB = O // P    # 16 output o-blocks
TS = 512        # token strip width
NS = T // TS    # 4 strips

F32 = mybir.dt.float32
BF16 = mybir.dt.bfloat16
FP8 = mybir.dt.float8e4

Alu = mybir.AluOpType
Act = mybir.ActivationFunctionType

_CACHE = {}


def _build(with_bias: bool):
    nc = bacc.Bacc("TRN2", target_bir_lowering=False, debug=False,
                   num_devices=N_CORES)
    xT_d = nc.dram_tensor("x", [D, T], F32, kind="ExternalInput").ap()
    w_d = nc.dram_tensor("W", [OSH, D], F32, kind="ExternalInput").ap()
    # y is stored transposed [O, T]; host transposes back
    y_d = nc.dram_tensor("y", [O, T], BF16, kind="ExternalOutput").ap()
    if with_bias:
        b_d = nc.dram_tensor("b", [O], F32, kind="ExternalInput").ap()

    # collective bounce buffers, one per local W block (half-shards);
    # payload = 2048 B of transposed code + 4 B of alpha per partition
    CCW = DP * 2 * P + 4  # 2052
    cc_in = [nc.dram_tensor(f"cc_in{k}", [P, CCW], FP8).ap()
             for k in range(NWL)]
    cc_out = [nc.dram_tensor(f"cc_out{k}", [N_CORES, P, CCW], FP8,
                             addr_space="Shared").ap()
              for k in range(NWL)]

    groups = [list(range(N_CORES))]

    with tile.TileContext(nc) as tc:
        with (
            tc.tile_pool(name="const", bufs=1) as const,
            tc.tile_pool(name="big", bufs=1) as big,
            tc.tile_pool(name="stats", bufs=1) as stats,
            tc.tile_pool(name="wload", bufs=2) as wload,
            tc.tile_pool(name="awc", bufs=2) as awc_pool,
            tc.tile_pool(name="cpos", bufs=2) as cpos_pool,
            tc.tile_pool(name="codem", bufs=2) as codem_pool,
            tc.tile_pool(name="xload", bufs=22) as xload,
            tc.tile_pool(name="junk", bufs=1) as junk_pool,
            tc.tile_pool(name="ystrip", bufs=6) as ystrip_pool,
            tc.tile_pool(name="psum_mm", bufs=6, space="PSUM") as psum_mm,
            tc.tile_pool(name="psum_tx", bufs=2, space="PSUM") as psum_tx,
        ):
            # ---- W loads first so the quant->AllGather chain starts asap
            # (scalar = ACT hardware DGE ring; W bytes lead that ring)
            wt = [wload.tile([P, D], F32, name="wt", tag="wt")
                  for k in range(NWL)]
            for k in range(NWL):
                nc.scalar.dma_start(wt[k][:], w_d[k * P:(k + 1) * P, :])

            id_bf = const.tile([P, P], BF16, tag="id_bf")
            make_identity(nc, id_bf[:])

            # fp8 operand tensors
            xT8 = big.tile([P, DP, 2, T], FP8, tag="xT8")
            codeT_sh = big.tile([P, NWL, DP, 2, P], FP8, tag="codeT_sh")
            # gathered code: [p, k, c, dp, 2, j] -> for a fixed k the
            # [c, dp, 2, j] block is contiguous (one bulk DMA per chunk)
            codeT = big.tile([P, NWL, N_CORES, DP, 2, P], FP8, tag="codeT")
            # alpha'[, bias] indexed [k, c] to match the gather layout
            alpha_sb = big.tile([P, NWL, N_CORES], F32, tag="alpha_sb")
            if with_bias:
                bias_sb = big.tile([P, NWL, N_CORES], F32, tag="bias_sb")
                for k in range(NWL):
                    for c in range(N_CORES):
                        ob = c * NWL + k
                        nc.gpsimd.dma_start(
                            bias_sb[:, k, c:c + 1],
                            b_d[ob * P:(ob + 1) * P].rearrange(
                                "(p one) -> p one", one=1)[:, :])

            # per-row stats, one column per local W block
            S_all = stats.tile([P, NWL], F32, tag="S")
            negmean = stats.tile([P, NWL], F32, tag="negmean")
            T_all = stats.tile([P, NWL], F32, tag="T")
            thr = stats.tile([P, NWL], F32, tag="thr")
            negthr = stats.tile([P, NWL], F32, tag="negthr")
            hi = stats.tile([P, NWL], F32, tag="hi")
            lo = stats.tile([P, NWL], F32, tag="lo")
            npos = stats.tile([P, NWL], F32, tag="npos")
            cmacc = stats.tile([P, NWL], F32, tag="cmacc")
            R_all = stats.tile([P, NWL], F32, tag="R")
            den = stats.tile([P, NWL], F32, tag="den")
            num = stats.tile([P, NWL], F32, tag="num")
            denc = stats.tile([P, NWL], F32, tag="denc")
            rden = stats.tile([P, NWL], F32, tag="rden")
            alpha_m = stats.tile([P, NWL], F32, tag="alpha_m")

            junk = junk_pool.tile([P, D], BF16, tag="junk")

            # ---- x strip loads; tiles consumed in (strip, d) order
            xf32 = {}

            def x_load(s, di, engine):
                t = xload.tile([P, TS], F32, name="xs", tag="xs")
                engine.dma_start(
                    t[:], xT_d[di * P:(di + 1) * P, s * TS:(s + 1) * TS])
                xf32[(s, di)] = t

            def x_sign(s, di):
                # sign via one DVE pass: (x >= 0) - 0.5 in {+-0.5}; the
                # missing 2x lives in alpha'.
                t = xf32.pop((s, di))
                nc.vector.tensor_scalar(
                    out=xT8[:, di // 2, di % 2, s * TS:(s + 1) * TS],
                    in0=t[:], scalar1=0.0, scalar2=0.5,
                    op0=Alu.is_ge, op1=Alu.subtract)

            def w_quant(k):
                ks = slice(k, k + 1)
                # S = row sum (ACT Copy with accumulate, junk output)
                nc.scalar.activation(
                    out=junk[:], in_=wt[k][:], func=Act.Copy,
                    accum_out=S_all[:, ks])
                nc.scalar.mul(negmean[:, ks], S_all[:, ks], -1.0 / D)
                aWc = awc_pool.tile([P, D], F32)
                nc.scalar.activation(
                    out=aWc[:], in_=wt[k][:], func=Act.Abs,
                    bias=negmean[:, ks], accum_out=T_all[:, ks])
                nc.scalar.mul(thr[:, ks], T_all[:, ks], DELTA_W / D)
                nc.scalar.mul(negthr[:, ks], T_all[:, ks], -DELTA_W / D)
                nc.vector.tensor_sub(hi[:, ks], thr[:, ks], negmean[:, ks])
                nc.vector.tensor_sub(lo[:, ks], negthr[:, ks], negmean[:, ks])
                nc.scalar.activation(
                    out=junk[:], in_=aWc[:], func=Act.Relu,
                    bias=negthr[:, ks], accum_out=R_all[:, ks])
                cp = cpos_pool.tile([P, D], BF16)
                nc.vector.tensor_scalar(
                    out=cp[:], in0=wt[k][:], scalar1=hi[:, ks], scalar2=0.0,
                    op0=Alu.is_ge, op1=Alu.add, accum_out=npos[:, ks])
                cm = codem_pool.tile([P, D], BF16)
                nc.vector.scalar_tensor_tensor(
                    out=cm[:], in0=wt[k][:], scalar=lo[:, ks], in1=cp[:],
                    op0=Alu.is_le, op1=Alu.subtract,
                    accum_out=cmacc[:, ks])
                # transpose the 16 d-tiles of code_m through the PE (bf16)
                for g in range(2):
                    ps = psum_tx.tile([P, 8 * P], BF16)
                    for j in range(8):
                        di = 8 * g + j
                        nc.tensor.matmul(
                            ps[:, j * P:(j + 1) * P],
                            cm[:, di * P:(di + 1) * P],
                            id_bf[:], is_transpose=True)
                    nc.vector.tensor_copy(
                        out=codeT_sh[:, k, 4 * g:4 * g + 4, :, :],
                        in_=ps.rearrange("p (a b t) -> p a b t",
                                         a=4, b=2, t=P)[:, :, :, :])
                # alpha' = -2*alpha: cm holds -code (sign flip), and the
                # {+-0.5} activation encoding contributes the 2x
                nc.vector.scalar_tensor_tensor(
                    out=den[:, ks], in0=npos[:, ks], scalar=2.0,
                    in1=cmacc[:, ks], op0=Alu.mult, op1=Alu.add)
                nc.vector.tensor_mul(num[:, ks], thr[:, ks], den[:, ks])
                nc.vector.tensor_add(num[:, ks], num[:, ks], R_all[:, ks])
                nc.vector.tensor_scalar_max(denc[:, ks], den[:, ks], 1.0)
                nc.vector.reciprocal(rden[:, ks], denc[:, ks])
                nc.vector.scalar_tensor_tensor(
                    out=alpha_m[:, ks], in0=num[:, ks], scalar=-2.0,
                    in1=rden[:, ks], op0=Alu.mult, op1=Alu.mult)

            def cc_send(k):
                nc.sync.dma_start(
                    cc_in[k][:, 0:DP * 2 * P].rearrange(
                        "p (a b j) -> p a b j", a=DP, b=2, j=P)[:, :, :, :],
                    codeT_sh[:, k, :, :, :])
                nc.sync.dma_start(
                    cc_in[k][:, DP * 2 * P:].bitcast(F32),
                    alpha_m[:, k:k + 1])
                nc.gpsimd.collective_compute(
                    "AllGather", Alu.bypass, replica_groups=groups,
                    ins=[cc_in[k][:, :].opt()],
                    outs=[cc_out[k][:, :, :].opt()])

            def cc_recv(k):
                # one bulk code read + one alpha read per chunk
                nc.sync.dma_start(
                    codeT[:, k, :, :, :, :].rearrange(
                        "p c a b j -> p c (a b j)")[:, :, :],
                    cc_out[k][:, :, 0:DP * 2 * P].rearrange(
                        "c p j -> p c j")[:, :, :])
                nc.sync.dma_start(
                    alpha_sb[:, k, :],
                    cc_out[k][:, :, DP * 2 * P:].bitcast(F32).rearrange(
                        "c p one -> p (c one)")[:, :])

            def mm_block(s, k, c):
                """y^T[(2c+k)*128:(2c+k+1)*128, s*TS:(s+1)*TS]."""
                ob = c * NWL + k
                ps = psum_mm.tile([P, TS], F32, tag="mmps")
                for dp in range(DP):
                    nc.tensor.matmul(
                        ps[:],
                        codeT[:, k, c, dp, :, :],
                        xT8[:, dp, :, s * TS:(s + 1) * TS],
                        start=(dp == 0), stop=(dp == DP - 1),
                        perf_mode=mybir.MatmulPerfMode.DoubleRow)
                ysT = ystrip_pool.tile([P, TS], BF16)
                # y^T = psum * alpha'_m (+ bias), per-partition scale on ACT
                if with_bias:
                    nc.scalar.activation(
                        out=ysT[:], in_=ps[:], func=Act.Copy,
                        scale=alpha_sb[:, k, c:c + 1],
                        bias=bias_sb[:, k, c:c + 1])
                else:
                    nc.scalar.activation(
                        out=ysT[:], in_=ps[:], func=Act.Copy,
                        scale=alpha_sb[:, k, c:c + 1])
                nc.scalar.dma_start(
                    y_d[ob * P:(ob + 1) * P, s * TS:(s + 1) * TS], ysT[:])

            # ---- emission ----
            # x strips 0/1 d0-7 on the scalar ring (bytes follow W);
            # strip 0 d8-15 lead the sync ring
            for s in (0, 1):
                for di in range(8):
                    x_load(s, di, nc.scalar)
            for di in range(8, 16):
                x_load(0, di, nc.sync)

            w_quant(0)
            cc_send(0)
            w_quant(1)
            cc_send(1)

            for di in range(8, 16):
                x_load(1, di, nc.sync)

            cc_recv(0)
            cc_recv(1)

            for di in range(16):
                x_sign(0, di)

            for s in (2, 3):
                for di in range(16):
                    x_load(s, di, nc.sync)

            for s in range(NS):
                if s >= 1:
                    for di in range(16):
                        x_sign(s, di)
                for k in range(NWL):
                    for c in range(N_CORES):
                        mm_block(s, k, c)

    nc.compile()
    return nc


def _get_nc(with_bias: bool):
    key = with_bias
    if key not in _CACHE:
        _CACHE[key] = _build(with_bias)
    return _CACHE[key]


def _make_in_maps(x, W, b, with_bias):
    xf = np.ascontiguousarray(x.reshape(B * S, D))
    in_maps = []
    for c in range(N_CORES):
        m = {"x": np.ascontiguousarray(xf[c * T:(c + 1) * T].T),
             "W": np.ascontiguousarray(W[c * OSH:(c + 1) * OSH])}
        if with_bias:
            m["b"] = b
        in_maps.append(m)
    return in_maps


def kernel(x: np.ndarray, W: np.ndarray, b: np.ndarray) -> np.ndarray:
    x = np.asarray(x, dtype=np.float32)
    W = np.ascontiguousarray(W, dtype=np.float32)
    b = np.asarray(b, dtype=np.float32)
    with_bias = bool(np.any(b))

    nc = _get_nc(with_bias)
    in_maps = _make_in_maps(x, W, b, with_bias)

    res = run_bass_kernel_spmd(nc, in_maps, core_ids=list(range(N_CORES)))
    # per-core y is [O, T]; transpose back and stack along tokens
    y = np.concatenate(
        [np.asarray(res.results[c]["y"]).astype(np.float32).T
         for c in range(N_CORES)], axis=0)
    return np.ascontiguousarray(y.reshape(B, S, O))


if __name__ == "__main__":
    rng = np.random.default_rng(0)
    x = rng.standard_normal((B, S, D), dtype=np.float32)
    W = rng.standard_normal((O, D), dtype=np.float32) * 0.03
    b = np.zeros((O,), dtype=np.float32)
    y = kernel(x, W, b)
    print("kernel ran, y shape", y.shape, "mean|y|", np.abs(y).mean())


# revision 7
# speedup vs baseline: 1.0504x; 1.0504x over previous
"""BitLinear forward kernel for Trainium2 (8 NeuronCores) — v10.

Structure (per core, tokens sharded 8-way, W-quant sharded 8-way):
- host pre-transposes x (layout only): device gets xT [D, T] f32
- W shard (256 rows) loads first on the ACT hardware-DGE ring; the
  ternary quant stats run on ACT/DVE, code is PE-transposed, and the
  fp8 code+alpha payload AllGathers in two 128-row chunks
- a dep-free dummy AllGather fires at kernel start to absorb the ncfw
  TOPSP cold-start so the real gathers run warm
- x streams as 4 token-strips of 512: strips 0 on the sync ring,
  1 on the scalar ring (behind W), strips 2/3 on the scalar ring
  behind the gather bounce-reads — keeping HBM quiet during the
  AllGather window; one DMA per (strip, d-half) = 8 triggers total
- sign(x) is one DVE pass to {-0.5,+0.5} fp8; the missing 2x is folded
  into alpha' = -2*alpha
- mm: stationary code chunk [128d-pair, 128o], moving sign(x)^T, fp8
  DoubleRow into psum [128, 512] accumulated over 8 d-pairs; ACT
  evicts with per-partition scale alpha' (+bias) to bf16; y^T stores
  batch 8 o-blocks per DMA
"""

import sys

for _p in ("/opt/trn_rl_repo", "/opt/trn_rl_repo/concourse"):
    if _p not in sys.path:
        sys.path.insert(0, _p)

import numpy as np

import concourse.bass as bass
import concourse.tile as tile
import concourse.mybir as mybir
from concourse import bacc
from concourse.bass_utils import run_bass_kernel_spmd
from concourse.masks import make_identity

# Problem shape (hardcoded per contract)
B, S, D, O = 4, 4096, 2048, 2048
N_CORES = 8
T = (B * S) // N_CORES  # 2048 token rows per core
OSH = O // N_CORES      # 256 W rows quantized per core
DELTA_W = 0.05

P = 128
NWL = OSH // P  # 2 local W blocks
ND = D // P     # 16 d-tiles
DP = ND // 2    # 8 DoubleRow d-pairs
NOB = O // P    # 16 output o-blocks
TS = 512        # token strip width
NS = T // TS    # 4 strips

F32 = mybir.dt.float32
BF16 = mybir.dt.bfloat16
FP8 = mybir.dt.float8e4

Alu = mybir.AluOpType
Act = mybir.ActivationFunctionType

_CACHE = {}


def _build(with_bias: bool):
    nc = bacc.Bacc("TRN2", target_bir_lowering=False, debug=False,
                   num_devices=N_CORES)
    xT_d = nc.dram_tensor("x", [D, T], F32, kind="ExternalInput").ap()
    w_d = nc.dram_tensor("W", [OSH, D], F32, kind="ExternalInput").ap()
    # y is stored transposed [O, T]; host transposes back
    y_d = nc.dram_tensor("y", [O, T], BF16, kind="ExternalOutput").ap()
    if with_bias:
        b_d = nc.dram_tensor("b", [O], F32, kind="ExternalInput").ap()

    # collective bounce buffers, one per local W block (half-shards);
    # payload = 2048 B of transposed code + 4 B of alpha per partition
    CCW = DP * 2 * P + 4  # 2052
    cc_in = [nc.dram_tensor(f"cc_in{k}", [P, CCW], FP8).ap()
             for k in range(NWL)]
    cc_out = [nc.dram_tensor(f"cc_out{k}", [N_CORES, P, CCW], FP8,
                             addr_space="Shared").ap()
              for k in range(NWL)]
    # dummy warmup collective (contents irrelevant, no producers)
    warm_in = nc.dram_tensor("warm_in", [1, 128], FP8).ap()
    warm_out = nc.dram_tensor("warm_out", [N_CORES, 1, 128], FP8,
                              addr_space="Shared").ap()

    groups = [list(range(N_CORES))]

    with tile.TileContext(nc) as tc:
        with (
            tc.tile_pool(name="const", bufs=1) as const,
            tc.tile_pool(name="big", bufs=1) as big,
            tc.tile_pool(name="stats", bufs=1) as stats,
            tc.tile_pool(name="wload", bufs=2) as wload,
            tc.tile_pool(name="awc", bufs=2) as awc_pool,
            tc.tile_pool(name="cpos", bufs=2) as cpos_pool,
            tc.tile_pool(name="codem", bufs=2) as codem_pool,
            tc.tile_pool(name="xload", bufs=3) as xload,
            tc.tile_pool(name="junk", bufs=1) as junk_pool,
            tc.tile_pool(name="ystrip", bufs=2) as ystrip_pool,
            tc.tile_pool(name="psum_mm", bufs=6, space="PSUM") as psum_mm,
            tc.tile_pool(name="psum_tx", bufs=2, space="PSUM") as psum_tx,
        ):
            # warmup AllGather: no data deps, fires right after preamble
            nc.gpsimd.collective_compute(
                "AllGather", Alu.bypass, replica_groups=groups,
                ins=[warm_in[:, :].opt()],
                outs=[warm_out[:, :, :].opt()])

            # W loads lead the ACT hardware-DGE ring
            wt = [wload.tile([P, D], F32, name="wt", tag="wt")
                  for k in range(NWL)]
            for k in range(NWL):
                nc.scalar.dma_start(wt[k][:], w_d[k * P:(k + 1) * P, :])

            id_bf = const.tile([P, P], BF16, tag="id_bf")
            make_identity(nc, id_bf[:])

            # fp8 operand tensors
            xT8 = big.tile([P, DP, 2, T], FP8, tag="xT8")
            codeT_sh = big.tile([P, NWL, DP, 2, P], FP8, tag="codeT_sh")
            # gathered code: [p, k, c, dp, 2, j] -> for a fixed k the
            # [c, dp, 2, j] block is contiguous (one bulk DMA per chunk)
            codeT = big.tile([P, NWL, N_CORES, DP, 2, P], FP8, tag="codeT")
            # alpha'[, bias] indexed [k, c] to match the gather layout
            alpha_sb = big.tile([P, NWL, N_CORES], F32, tag="alpha_sb")
            if with_bias:
                bias_sb = big.tile([P, NWL, N_CORES], F32, tag="bias_sb")

            # per-row stats, one column per local W block
            S_all = stats.tile([P, NWL], F32, tag="S")
            negmean = stats.tile([P, NWL], F32, tag="negmean")
            T_all = stats.tile([P, NWL], F32, tag="T")
            thr = stats.tile([P, NWL], F32, tag="thr")
            negthr = stats.tile([P, NWL], F32, tag="negthr")
            hi = stats.tile([P, NWL], F32, tag="hi")
            lo = stats.tile([P, NWL], F32, tag="lo")
            npos = stats.tile([P, NWL], F32, tag="npos")
            cmacc = stats.tile([P, NWL], F32, tag="cmacc")
            R_all = stats.tile([P, NWL], F32, tag="R")
            den = stats.tile([P, NWL], F32, tag="den")
            num = stats.tile([P, NWL], F32, tag="num")
            denc = stats.tile([P, NWL], F32, tag="denc")
            rden = stats.tile([P, NWL], F32, tag="rden")
            alpha_m = stats.tile([P, NWL], F32, tag="alpha_m")

            junk = junk_pool.tile([P, D], BF16, tag="junk")

            # ---- x half-strip loads: one DMA covers 8 d-tiles
            xf32 = {}

            def x_load(s, h, engine):
                # tile [p, di(8), t]: per-partition contiguous 16 KB
                t = xload.tile([P, 8, TS], F32, name="xs", tag="xs")
                engine.dma_start(
                    t[:],
                    xT_d.rearrange("(dd p) t -> p dd t", p=P)[
                        :, 8 * h:8 * h + 8, s * TS:(s + 1) * TS])
                xf32[(s, h)] = t

            def x_sign(s, h):
                # sign via one DVE pass per d-tile: (x>=0) - 0.5 in
                # {+-0.5}; the missing 2x lives in alpha'.
                t = xf32.pop((s, h))
                for j in range(8):
                    di = 8 * h + j
                    nc.vector.tensor_scalar(
                        out=xT8[:, di // 2, di % 2, s * TS:(s + 1) * TS],
                        in0=t[:, j, :], scalar1=0.0, scalar2=0.5,
                        op0=Alu.is_ge, op1=Alu.subtract)

            def w_quant(k):
                ks = slice(k, k + 1)
                # S = row sum (ACT Copy with accumulate, junk output)
                nc.scalar.activation(
                    out=junk[:], in_=wt[k][:], func=Act.Copy,
                    accum_out=S_all[:, ks])
                nc.scalar.mul(negmean[:, ks], S_all[:, ks], -1.0 / D)
                aWc = awc_pool.tile([P, D], F32)
                nc.scalar.activation(
                    out=aWc[:], in_=wt[k][:], func=Act.Abs,
                    bias=negmean[:, ks], accum_out=T_all[:, ks])
                nc.scalar.mul(thr[:, ks], T_all[:, ks], DELTA_W / D)
                nc.scalar.mul(negthr[:, ks], T_all[:, ks], -DELTA_W / D)
                nc.vector.tensor_sub(hi[:, ks], thr[:, ks], negmean[:, ks])
                nc.vector.tensor_sub(lo[:, ks], negthr[:, ks], negmean[:, ks])
                nc.scalar.activation(
                    out=junk[:], in_=aWc[:], func=Act.Relu,
                    bias=negthr[:, ks], accum_out=R_all[:, ks])
                cp = cpos_pool.tile([P, D], BF16)
                nc.vector.tensor_scalar(
                    out=cp[:], in0=wt[k][:], scalar1=hi[:, ks], scalar2=0.0,
                    op0=Alu.is_ge, op1=Alu.add, accum_out=npos[:, ks])
                cm = codem_pool.tile([P, D], BF16)
                nc.vector.scalar_tensor_tensor(
                    out=cm[:], in0=wt[k][:], scalar=lo[:, ks], in1=cp[:],
                    op0=Alu.is_le, op1=Alu.subtract,
                    accum_out=cmacc[:, ks])
                # transpose the 16 d-tiles of code_m through the PE (bf16)
                for g in range(2):
                    ps = psum_tx.tile([P, 8 * P], BF16)
                    for j in range(8):
                        di = 8 * g + j
                        nc.tensor.matmul(
                            ps[:, j * P:(j + 1) * P],
                            cm[:, di * P:(di + 1) * P],
                            id_bf[:], is_transpose=True)
                    nc.vector.tensor_copy(
                        out=codeT_sh[:, k, 4 * g:4 * g + 4, :, :],
                        in_=ps.rearrange("p (a b t) -> p a b t",
                                         a=4, b=2, t=P)[:, :, :, :])
                # alpha' = -2*alpha: cm holds -code (sign flip), and the
                # {+-0.5} activation encoding contributes the 2x
                nc.vector.scalar_tensor_tensor(
                    out=den[:, ks], in0=npos[:, ks], scalar=2.0,
                    in1=cmacc[:, ks], op0=Alu.mult, op1=Alu.add)
                nc.vector.tensor_mul(num[:, ks], thr[:, ks], den[:, ks])
                nc.vector.tensor_add(num[:, ks], num[:, ks], R_all[:, ks])
                nc.vector.tensor_scalar_max(denc[:, ks], den[:, ks], 1.0)
                nc.vector.reciprocal(rden[:, ks], denc[:, ks])
                nc.vector.scalar_tensor_tensor(
                    out=alpha_m[:, ks], in0=num[:, ks], scalar=-2.0,
                    in1=rden[:, ks], op0=Alu.mult, op1=Alu.mult)

            def cc_send(k):
                # payload writes ride the idle SWDGE ring (gpsimd), which
                # also triggers the collective right after
                nc.gpsimd.dma_start(
                    cc_in[k][:, 0:DP * 2 * P].rearrange(
                        "p (a b j) -> p a b j", a=DP, b=2, j=P)[:, :, :, :],
                    codeT_sh[:, k, :, :, :])
                nc.gpsimd.dma_start(
                    cc_in[k][:, DP * 2 * P:].bitcast(F32),
                    alpha_m[:, k:k + 1])
                nc.gpsimd.collective_compute(
                    "AllGather", Alu.bypass, replica_groups=groups,
                    ins=[cc_in[k][:, :].opt()],
                    outs=[cc_out[k][:, :, :].opt()])

            def cc_recv(k):
                # bounce reads ride the (idle) ACT ring: one bulk code
                # read + one alpha read per chunk
                nc.scalar.dma_start(
                    codeT[:, k, :, :, :, :].rearrange(
                        "p c a b j -> p c (a b j)")[:, :, :],
                    cc_out[k][:, :, 0:DP * 2 * P].rearrange(
                        "c p j -> p c j")[:, :, :])
                nc.scalar.dma_start(
                    alpha_sb[:, k, :],
                    cc_out[k][:, :, DP * 2 * P:].bitcast(F32).rearrange(
                        "c p one -> p (c one)")[:, :])

            def mm_wave(s, k):
                """y^T rows ob=c*2+k for all c, strip s."""
                ysT = ystrip_pool.tile([P, N_CORES, TS], BF16,
                                       name="ysT", tag="ysT")
                for c in range(N_CORES):
                    ps = psum_mm.tile([P, TS], F32, tag="mmps")
                    for dp in range(DP):
                        nc.tensor.matmul(
                            ps[:],
                            codeT[:, k, c, dp, :, :],
                            xT8[:, dp, :, s * TS:(s + 1) * TS],
                            start=(dp == 0), stop=(dp == DP - 1),
                            perf_mode=mybir.MatmulPerfMode.DoubleRow)
                    # y^T = psum * alpha' (+ bias): per-partition scale
                    if with_bias:
                        nc.scalar.activation(
                            out=ysT[:, c, :], in_=ps[:], func=Act.Copy,
                            scale=alpha_sb[:, k, c:c + 1],
                            bias=bias_sb[:, k, c:c + 1])
                    else:
                        nc.scalar.activation(
                            out=ysT[:, c, :], in_=ps[:], func=Act.Copy,
                            scale=alpha_sb[:, k, c:c + 1])
                # one strided store for the 8 o-blocks of this (k, s)
                nc.scalar.dma_start(
                    y_d.rearrange("(c kk p) t -> p c kk t", p=P, kk=NWL)[
                        :, :, k, s * TS:(s + 1) * TS],
                    ysT[:, :, :])

            # ---- emission ----
            # x strip 0 on the sync ring; strip 1 on the scalar ring
            # (bytes follow W); strips 2/3 later on the scalar ring
            for h in (0, 1):
                x_load(0, h, nc.sync)
            for h in (0, 1):
                x_load(1, h, nc.scalar)

            with tc.high_priority():
                w_quant(0)
                cc_send(0)
                w_quant(1)
                cc_send(1)
                cc_recv(0)
                cc_recv(1)

            if with_bias:
                for k in range(NWL):
                    for c in range(N_CORES):
                        ob = c * NWL + k
                        nc.gpsimd.dma_start(
                            bias_sb[:, k, c:c + 1],
                            b_d[ob * P:(ob + 1) * P].rearrange(
                                "(p one) -> p one", one=1)[:, :])

            x_sign(0, 0)
            x_sign(0, 1)
            x_sign(1, 0)
            x_sign(1, 1)

            # strips 2/3 load behind the bounce reads on the scalar ring
            # (keeps HBM quiet while the AllGathers are in flight)
            for s in (2, 3):
                for h in (0, 1):
                    x_load(s, h, nc.scalar)

            for s in range(NS):
                if s >= 2:
                    x_sign(s, 0)
                    x_sign(s, 1)
                for k in range(NWL):
                    mm_wave(s, k)

    nc.compile()
    return nc


def _get_nc(with_bias: bool):
    key = with_bias
    if key not in _CACHE:
        _CACHE[key] = _build(with_bias)
    return _CACHE[key]


def _make_in_maps(x, W, b, with_bias):
    xf = np.ascontiguousarray(x.reshape(B * S, D))
    in_maps = []
    for c in range(N_CORES):
        m = {"x": np.ascontiguousarray(xf[c * T:(c + 1) * T].T),
             "W": np.ascontiguousarray(W[c * OSH:(c + 1) * OSH])}
        if with_bias:
            m["b"] = b
        in_maps.append(m)
    return in_maps


def kernel(x: np.ndarray, W: np.ndarray, b: np.ndarray) -> np.ndarray:
    x = np.asarray(x, dtype=np.float32)
    W = np.ascontiguousarray(W, dtype=np.float32)
    b = np.asarray(b, dtype=np.float32)
    with_bias = bool(np.any(b))

    nc = _get_nc(with_bias)
    in_maps = _make_in_maps(x, W, b, with_bias)

    res = run_bass_kernel_spmd(nc, in_maps, core_ids=list(range(N_CORES)))
    # per-core y is [O, T]; transpose back and stack along tokens
    y = np.concatenate(
        [np.asarray(res.results[c]["y"]).astype(np.float32).T
         for c in range(N_CORES)], axis=0)
    return np.ascontiguousarray(y.reshape(B, S, O))


if __name__ == "__main__":
    rng = np.random.default_rng(0)
    x = rng.standard_normal((B, S, D), dtype=np.float32)
    W = rng.standard_normal((O, D), dtype=np.float32) * 0.03
    b = np.zeros((O,), dtype=np.float32)
    y = kernel(x, W, b)
    print("kernel ran, y shape", y.shape, "mean|y|", np.abs(y).mean())


# revision 10
# speedup vs baseline: 1.1293x; 1.0751x over previous
"""BitLinear forward kernel for Trainium2 (8 NeuronCores) — v11.

Per core (tokens sharded 8-way, W-quant sharded 8-way):
- host pre-transposes x (layout only): device gets xT [D, T] f32
- a dep-free dummy AllGather fires right after the NEFF preamble to
  absorb the ncfw/TOPSP cold-start, so the real gathers run warm
- W shard (2 blocks of 128 rows) leads the ACT hardware-DGE ring; the
  quant stats run on ACT (row-sum / |.| / relu accumulators), the
  ternary compare+code build on DVE, code is PE-transposed and the
  fp8 code+alpha payload AllGathers per block (2 chunks)
- x streams as 2 token-halves of 1024 on the sync ring (one DMA per
  (half, d-half): 4 total); sign(x) is one DVE pass per d-tile to
  {-0.5,+0.5} fp8 — gated behind the quant's DVE ops by a constant
  "gate" operand so the list scheduler cannot starve the quant chain
- mm: stationary code chunk [128dp, 2, 128o] fp8 DoubleRow, moving
  sign(x)^T strips of 512, psum [128, 1024] accumulated over 8 dp;
  ACT evicts halves with per-partition scale alpha' = -2*alpha
  (+bias) to bf16; y^T stores batch 8 o-blocks per DMA
"""

import sys

for _p in ("/opt/trn_rl_repo", "/opt/trn_rl_repo/concourse"):
    if _p not in sys.path:
        sys.path.insert(0, _p)

import numpy as np

import concourse.bass as bass
import concourse.tile as tile
import concourse.mybir as mybir
from concourse import bacc
from concourse.bass_utils import run_bass_kernel_spmd
from concourse.masks import make_identity

# Problem shape (hardcoded per contract)
B, S, D, O = 4, 4096, 2048, 2048
N_CORES = 8
T = (B * S) // N_CORES  # 2048 token rows per core
OSH = O // N_CORES      # 256 W rows quantized per core
DELTA_W = 0.05

P = 128
NWL = OSH // P  # 2 local W blocks
ND = D // P     # 16 d-tiles
DP = ND // 2    # 8 DoubleRow d-pairs
NOB = O // P    # 16 output o-blocks
TS = 1024       # token strip width (one x half)
NS = T // TS    # 2 strips
TQ = 512        # matmul moving width (DoubleRow cap)

F32 = mybir.dt.float32
BF16 = mybir.dt.bfloat16
FP8 = mybir.dt.float8e4

Alu = mybir.AluOpType
Act = mybir.ActivationFunctionType

_CACHE = {}


def _build(with_bias: bool):
    nc = bacc.Bacc("TRN2", target_bir_lowering=False, debug=False,
                   num_devices=N_CORES)
    xT_d = nc.dram_tensor("x", [D, T], F32, kind="ExternalInput").ap()
    w_d = nc.dram_tensor("W", [OSH, D], F32, kind="ExternalInput").ap()
    # y is stored transposed [O, T]; host transposes back
    y_d = nc.dram_tensor("y", [O, T], BF16, kind="ExternalOutput").ap()
    if with_bias:
        b_d = nc.dram_tensor("b", [O], F32, kind="ExternalInput").ap()

    # collective bounce buffers, one per local W block (half-shards);
    # payload = 2048 B of transposed code + 4 B of alpha per partition
    CCW = DP * 2 * P + 4  # 2052
    cc_in = [nc.dram_tensor(f"cc_in{k}", [P, CCW], FP8).ap()
             for k in range(NWL)]
    cc_out = [nc.dram_tensor(f"cc_out{k}", [N_CORES, P, CCW], FP8,
                             addr_space="Shared").ap()
              for k in range(NWL)]
    # dummy warmup collective (contents irrelevant, no producers)
    warm_in = nc.dram_tensor("warm_in", [1, 128], FP8).ap()
    warm_out = nc.dram_tensor("warm_out", [N_CORES, 1, 128], FP8,
                              addr_space="Shared").ap()

    groups = [list(range(N_CORES))]

    with tile.TileContext(nc) as tc:
        with (
            tc.tile_pool(name="const", bufs=1) as const,
            tc.tile_pool(name="big", bufs=1) as big,
            tc.tile_pool(name="stats", bufs=1) as stats,
            tc.tile_pool(name="wload", bufs=2) as wload,
            tc.tile_pool(name="awc", bufs=2) as awc_pool,
            tc.tile_pool(name="cpos", bufs=2) as cpos_pool,
            tc.tile_pool(name="codem", bufs=2) as codem_pool,
            tc.tile_pool(name="xload", bufs=2) as xload,
            tc.tile_pool(name="junk", bufs=1) as junk_pool,
            tc.tile_pool(name="ystrip", bufs=2) as ystrip_pool,
            tc.tile_pool(name="psum_mm", bufs=3, space="PSUM") as psum_mm,
            tc.tile_pool(name="psum_tx", bufs=2, space="PSUM") as psum_tx,
        ):
            # warmup AllGather: no data deps, fires right after preamble
            nc.gpsimd.collective_compute(
                "AllGather", Alu.bypass, replica_groups=groups,
                ins=[warm_in[:, :].opt()],
                outs=[warm_out[:, :, :].opt()])

            # W loads lead the ACT hardware-DGE ring
            wt = [wload.tile([P, D], F32, name="wt", tag="wt")
                  for k in range(NWL)]
            for k in range(NWL):
                nc.scalar.dma_start(wt[k][:], w_d[k * P:(k + 1) * P, :])

            id_bf = const.tile([P, P], BF16, tag="id_bf")
            make_identity(nc, id_bf[:])

            # fp8 operand tensors
            xT8 = big.tile([P, DP, 2, T], FP8, tag="xT8")
            codeT_sh = big.tile([P, NWL, DP, 2, P], FP8, tag="codeT_sh")
            codeT = big.tile([P, NWL, N_CORES, DP, 2, P], FP8, tag="codeT")
            alpha_sb = big.tile([P, NWL, N_CORES], F32, tag="alpha_sb")
            gate = big.tile([P, TS], F32, tag="gate")
            if with_bias:
                bias_sb = big.tile([P, NWL, N_CORES], F32, tag="bias_sb")

            # per-row stats, one column per local W block
            S_all = stats.tile([P, NWL], F32, tag="S")
            negmean = stats.tile([P, NWL], F32, tag="negmean")
            T_all = stats.tile([P, NWL], F32, tag="T")
            thr = stats.tile([P, NWL], F32, tag="thr")
            negthr = stats.tile([P, NWL], F32, tag="negthr")
            hi = stats.tile([P, NWL], F32, tag="hi")
            lo = stats.tile([P, NWL], F32, tag="lo")
            npos = stats.tile([P, NWL], F32, tag="npos")
            cmacc = stats.tile([P, NWL], F32, tag="cmacc")
            R_all = stats.tile([P, NWL], F32, tag="R")
            den = stats.tile([P, NWL], F32, tag="den")
            num = stats.tile([P, NWL], F32, tag="num")
            denc = stats.tile([P, NWL], F32, tag="denc")
            rden = stats.tile([P, NWL], F32, tag="rden")
            alpha_m = stats.tile([P, NWL], F32, tag="alpha_m")

            junk = junk_pool.tile([P, D], BF16, tag="junk")

            # ---- quant: ACT stats passes, interleaved across the two
            # blocks so the accumulator read gaps pipeline
            def q_sum(k):
                ks = slice(k, k + 1)
                nc.scalar.activation(
                    out=junk[:], in_=wt[k][:], func=Act.Copy,
                    accum_out=S_all[:, ks])
                nc.scalar.mul(negmean[:, ks], S_all[:, ks], -1.0 / D)

            aWc = {}

            def q_abs(k):
                ks = slice(k, k + 1)
                a = awc_pool.tile([P, D], BF16, name="aWc", tag="aWc")
                nc.scalar.activation(
                    out=a[:], in_=wt[k][:], func=Act.Abs,
                    bias=negmean[:, ks], accum_out=T_all[:, ks])
                aWc[k] = a
                nc.scalar.mul(thr[:, ks], T_all[:, ks], DELTA_W / D)
                nc.scalar.mul(negthr[:, ks], T_all[:, ks], -DELTA_W / D)
                nc.vector.tensor_sub(hi[:, ks], thr[:, ks], negmean[:, ks])
                nc.vector.tensor_sub(lo[:, ks], negthr[:, ks], negmean[:, ks])

            def q_relu(k):
                ks = slice(k, k + 1)
                nc.scalar.activation(
                    out=junk[:], in_=aWc.pop(k)[:], func=Act.Relu,
                    bias=negthr[:, ks], accum_out=R_all[:, ks])

            def q_code(k):
                ks = slice(k, k + 1)
                cp = cpos_pool.tile([P, D], BF16, name="cp", tag="cp")
                nc.vector.tensor_scalar(
                    out=cp[:], in0=wt[k][:], scalar1=hi[:, ks], scalar2=0.0,
                    op0=Alu.is_ge, op1=Alu.add, accum_out=npos[:, ks])
                cm = codem_pool.tile([P, D], BF16, name="cm", tag="cm")
                nc.vector.scalar_tensor_tensor(
                    out=cm[:], in0=wt[k][:], scalar=lo[:, ks], in1=cp[:],
                    op0=Alu.is_le, op1=Alu.subtract,
                    accum_out=cmacc[:, ks])
                # transpose the 16 d-tiles of code_m through the PE (bf16)
                for g in range(2):
                    ps = psum_tx.tile([P, 8 * P], BF16)
                    for j in range(8):
                        di = 8 * g + j
                        nc.tensor.matmul(
                            ps[:, j * P:(j + 1) * P],
                            cm[:, di * P:(di + 1) * P],
                            id_bf[:], is_transpose=True)
                    nc.vector.tensor_copy(
                        out=codeT_sh[:, k, 4 * g:4 * g + 4, :, :],
                        in_=ps.rearrange("p (a b t) -> p a b t",
                                         a=4, b=2, t=P)[:, :, :, :])
                # alpha' = -2*alpha: cm holds -code (sign flip), and the
                # {+-0.5} activation encoding contributes the 2x
                nc.vector.scalar_tensor_tensor(
                    out=den[:, ks], in0=npos[:, ks], scalar=2.0,
                    in1=cmacc[:, ks], op0=Alu.mult, op1=Alu.add)
                nc.vector.tensor_mul(num[:, ks], thr[:, ks], den[:, ks])
                nc.vector.tensor_add(num[:, ks], num[:, ks], R_all[:, ks])
                nc.vector.tensor_scalar_max(denc[:, ks], den[:, ks], 1.0)
                nc.vector.reciprocal(rden[:, ks], denc[:, ks])
                nc.vector.scalar_tensor_tensor(
                    out=alpha_m[:, ks], in0=num[:, ks], scalar=-2.0,
                    in1=rden[:, ks], op0=Alu.mult, op1=Alu.mult)

            def cc_send(k):
                # payload writes + doorbell on the idle SWDGE ring
                nc.gpsimd.dma_start(
                    cc_in[k][:, 0:DP * 2 * P].rearrange(
                        "p (a b j) -> p a b j", a=DP, b=2, j=P)[:, :, :, :],
                    codeT_sh[:, k, :, :, :])
                nc.gpsimd.dma_start(
                    cc_in[k][:, DP * 2 * P:].bitcast(F32),
                    alpha_m[:, k:k + 1])
                nc.gpsimd.collective_compute(
                    "AllGather", Alu.bypass, replica_groups=groups,
                    ins=[cc_in[k][:, :].opt()],
                    outs=[cc_out[k][:, :, :].opt()])

            def cc_recv(k):
                # per-source reads so the first mm blocks start sooner;
                # they ride the (idle-by-then) ACT ring
                nc.scalar.dma_start(
                    alpha_sb[:, k, :],
                    cc_out[k][:, :, DP * 2 * P:].bitcast(F32).rearrange(
                        "c p one -> p (c one)")[:, :])
                for c in range(N_CORES):
                    nc.scalar.dma_start(
                        codeT[:, k, c, :, :, :],
                        cc_out[k][c, :, 0:DP * 2 * P].rearrange(
                            "p (a b j) -> p a b j",
                            a=DP, b=2, j=P)[:, :, :, :])

            # ---- emission: critical chain first (priority = order)
            q_sum(0)
            q_sum(1)
            q_abs(0)
            q_abs(1)
            q_relu(0)   # R feeds alpha, which rides the AllGather
            q_relu(1)
            q_code(0)
            cc_send(0)
            q_code(1)
            cc_send(1)
            cc_recv(0)
            cc_recv(1)

            # sign gate: constant 0.5, but its last column is rewritten
            # by an op that depends on the quant DVE chain — sign reads
            # of gate[:] therefore schedule after the quant (subtile
            # dependency), keeping the DVE free for the gather path
            nc.vector.memset(gate[:], 0.5)
            nc.vector.tensor_scalar(
                out=gate[:, TS - 1:TS], in0=rden[:, 1:2], scalar1=0.0,
                scalar2=0.5, op0=Alu.mult, op1=Alu.add)

            if with_bias:
                for k in range(NWL):
                    for c in range(N_CORES):
                        ob = c * NWL + k
                        nc.gpsimd.dma_start(
                            bias_sb[:, k, c:c + 1],
                            b_d[ob * P:(ob + 1) * P].rearrange(
                                "(p one) -> p one", one=1)[:, :])

            # ---- x halves: one DMA per (half, d-half) on the sync ring
            xf32 = {}

            def x_load(s, h):
                t = xload.tile([P, 8, TS], F32, name="xs", tag="xs")
                nc.sync.dma_start(
                    t[:],
                    xT_d.rearrange("(dd p) t -> p dd t", p=P)[
                        :, 8 * h:8 * h + 8, s * TS:(s + 1) * TS])
                xf32[(s, h)] = t

            def x_sign(s, h):
                # sign: (x>=0) - gate(=0.5) in {+-0.5}; one DVE pass per
                # d-tile; the gate operand orders signs after the quant
                t = xf32.pop((s, h))
                for j in range(8):
                    di = 8 * h + j
                    nc.vector.scalar_tensor_tensor(
                        out=xT8[:, di // 2, di % 2, s * TS:(s + 1) * TS],
                        in0=t[:, j, :], scalar=0.0, in1=gate[:],
                        op0=Alu.is_ge, op1=Alu.subtract)

            for s in range(NS):
                for h in (0, 1):
                    x_load(s, h)

            for h in (0, 1):
                x_sign(0, h)

            def mm_wave(s, k):
                """y^T rows ob=c*2+k for all c, strip s (width TS)."""
                for half in range(TS // TQ):
                    ysT = ystrip_pool.tile([P, N_CORES, TQ], BF16,
                                           name="ysT", tag="ysT")
                    t0 = s * TS + half * TQ
                    for c in range(N_CORES):
                        ps = psum_mm.tile([P, TQ], F32, tag="mmps")
                        for dp in range(DP):
                            nc.tensor.matmul(
                                ps[:],
                                codeT[:, k, c, dp, :, :],
                                xT8[:, dp, :, t0:t0 + TQ],
                                start=(dp == 0), stop=(dp == DP - 1),
                                perf_mode=mybir.MatmulPerfMode.DoubleRow)
                        if with_bias:
                            nc.scalar.activation(
                                out=ysT[:, c, :], in_=ps[:], func=Act.Copy,
                                scale=alpha_sb[:, k, c:c + 1],
                                bias=bias_sb[:, k, c:c + 1])
                        else:
                            nc.scalar.activation(
                                out=ysT[:, c, :], in_=ps[:], func=Act.Copy,
                                scale=alpha_sb[:, k, c:c + 1])
                    # one strided store for the 8 o-blocks of this half
                    nc.scalar.dma_start(
                        y_d.rearrange("(c kk p) t -> p c kk t",
                                      p=P, kk=NWL)[:, :, k, t0:t0 + TQ],
                        ysT[:, :, :])

            for s in range(NS):
                if s >= 1:
                    for h in (0, 1):
                        x_sign(s, h)
                for k in range(NWL):
                    mm_wave(s, k)

    nc.compile()
    return nc


def _get_nc(with_bias: bool):
    key = with_bias
    if key not in _CACHE:
        _CACHE[key] = _build(with_bias)
    return _CACHE[key]


def _make_in_maps(x, W, b, with_bias):
    xf = np.ascontiguousarray(x.reshape(B * S, D))
    in_maps = []
    for c in range(N_CORES):
        m = {"x": np.ascontiguousarray(xf[c * T:(c + 1) * T].T),
             "W": np.ascontiguousarray(W[c * OSH:(c + 1) * OSH])}
        if with_bias:
            m["b"] = b
        in_maps.append(m)
    return in_maps


def kernel(x: np.ndarray, W: np.ndarray, b: np.ndarray) -> np.ndarray:
    x = np.asarray(x, dtype=np.float32)
    W = np.ascontiguousarray(W, dtype=np.float32)
    b = np.asarray(b, dtype=np.float32)
    with_bias = bool(np.any(b))

    nc = _get_nc(with_bias)
    in_maps = _make_in_maps(x, W, b, with_bias)

    res = run_bass_kernel_spmd(nc, in_maps, core_ids=list(range(N_CORES)))
    # per-core y is [O, T]; transpose back and stack along tokens
    y = np.concatenate(
        [np.asarray(res.results[c]["y"]).astype(np.float32).T
         for c in range(N_CORES)], axis=0)
    return np.ascontiguousarray(y.reshape(B, S, O))


if __name__ == "__main__":
    rng = np.random.default_rng(0)
    x = rng.standard_normal((B, S, D), dtype=np.float32)
    W = rng.standard_normal((O, D), dtype=np.float32) * 0.03
    b = np.zeros((O,), dtype=np.float32)
    y = kernel(x, W, b)
    print("kernel ran, y shape", y.shape, "mean|y|", np.abs(y).mean())
